# revision 9
# baseline (speedup 1.0000x reference)
"""Masked linear (CantorLinear): y = x @ (weight*mask).T + bias.

Sparse flipped-orientation kernel. The Cantor mask keeps 326 contiguous
(row, k-run) pieces across 240 of the 2048 output rows. Each piece gets one
output "slot"; slots are sorted by run midpoint so that each 128-wide
k-subtile's user slots form a tight contiguous range (sum of ranges = 1529
vs 16*336 dense).

Per matmul: lhsT (stationary) = x k-subtile [128 k, 128 seq] in fp8-e3m4
(4-bit mantissa, abs err 1.2e-2 rel vs 2e-2 budget; halves x DMA vs fp16),
rhs (moving) = packed weight [128 k, range] in fp16, PSUM [128 seq, slots]
fp32 accumulates over the 16 k-subtiles. PE cost scales with the moving
free dim = slot range, so sparsity cuts PE ~2.2x vs the dense orientation.
PSUM has_written semantics (start=True clears the whole bank; start=False
overwrites where clear) make the per-subtile column offsets legal without
an init pass. Bias and the 2-piece row sums are applied host-side.

8 cores data-parallel over the 16384 sequence positions.
"""

import os
import numpy as np
import ml_dtypes

import concourse.bacc as bacc
import concourse.mybir as mybir
import concourse.tile as tile
import concourse.bass_utils as _bu
from concourse.bass_utils import run_bass_kernel_spmd

if os.environ.get("CANTOR_LDWOPT", "0") == "1" and not getattr(_bu, "_ldw_patched", False):
    _orig_run_command = _bu.run_command

    def _patched_run_command(argv, **kw):
        argv = ["--enable-ldw-opt=true" if a == "--enable-ldw-opt=false" else a
                for a in argv]
        return _orig_run_command(argv, **kw)

    _bu.run_command = _patched_run_command
    _bu._ldw_patched = True

B, SQ = 4, 4096
IN_F = 2048
OUT_F = 2048
S = B * SQ                 # 16384 flattened sequence positions
NCORES = 8
S_SH = S // NCORES         # 2048 per core
P = 128
KS = IN_F // P             # 16 k-subtiles
NT = int(os.environ.get("CANTOR_NT", "512"))   # seq positions per x DMA tile
NSI = S_SH // NT
SB = NT // P               # seq sub-blocks (psum groups) per x tile
SLOTS = 336                # 326 real slots padded

MM_MODE = os.environ.get("CANTOR_MM_MODE", "e3m4")
LOOPS = int(os.environ.get("CANTOR_BENCH_LOOPS", "1"))

LAST_RESULTS = None
_NC_CACHE = {}


def _cantor_mask(out_dim, in_dim, depth=8):
    idx = np.arange(out_dim * in_dim, dtype=np.float64)
    x = idx / (out_dim * in_dim + 1e-9)
    valid = np.ones(x.shape, dtype=bool)
    for _ in range(depth):
        x = x * 3.0
        digit = np.floor(x)
        x = x - digit
        valid &= (digit != 1.0)
    return valid.reshape(out_dim, in_dim)


def _build_tables():
    """Slot decomposition of the mask: returns (rows, slot list sorted by
    run midpoint, per-subtile [lo, hi) slot ranges, per-row slot indices)."""
    M = _cantor_mask(OUT_F, IN_F)
    rows = np.flatnonzero(M.any(axis=1))
    slots = []
    for i, r in enumerate(rows):
        m = M[r]
        d = np.diff(np.concatenate([[0], m.view(np.int8), [0]]))
        for s, e in zip(np.flatnonzero(d == 1), np.flatnonzero(d == -1)):
            slots.append((i, int(s), int(e)))
    slots.sort(key=lambda t: t[1] + t[2])
    assert len(slots) <= SLOTS
    lo = np.zeros(KS, np.int32)
    hi = np.zeros(KS, np.int32)
    for t in range(KS):
        a, b = t * P, (t + 1) * P
        idx = [j for j, (_, s, e) in enumerate(slots) if s < b and e > a]
        lo[t], hi[t] = idx[0], idx[-1] + 1
    first = np.full(len(rows), -1, np.int64)
    second = np.full(len(rows), -1, np.int64)
    for j, (i, s, e) in enumerate(slots):
        if first[i] < 0:
            first[i] = j
        else:
            second[i] = j
    return rows, slots, lo, hi, first, second


ROWS, SLOT_LIST, LO_T, HI_T, FIRST_SLOT, SECOND_SLOT = _build_tables()


XNAME = "xtl" if os.environ.get("CANTOR_LDWOPT", "0") == "1" else "xt"


def _build_nc(mm_mode, loops):
    x_dt = mybir.dt.float8e3
    nc = bacc.Bacc("TRN2", target_bir_lowering=False, debug=False)
    xt = nc.dram_tensor(XNAME, [NSI, P, KS, NT], x_dt, kind="ExternalInput")
    wt = nc.dram_tensor("wt", [KS, P, SLOTS], mybir.dt.float16,
                        kind="ExternalInput")
    # [si][sb][p][slot] — host reshapes to [S_SH, SLOTS] (same bytes)
    yt = nc.dram_tensor("yt", [NSI, SB, P, SLOTS], mybir.dt.float16,
                        kind="ExternalOutput")

    with tile.TileContext(nc) as tc:
        with (
            tc.tile_pool(name="wpool", bufs=1) as wpool,
            tc.tile_pool(name="xpool",
                         bufs=int(os.environ.get("CANTOR_XBUFS", "3"))) as xpool,
            tc.tile_pool(name="opool",
                         bufs=int(os.environ.get("CANTOR_OBUFS", "4"))) as opool,
            tc.tile_pool(name="pspool",
                         bufs=int(os.environ.get("CANTOR_PSBUFS", "4")),
                         space="PSUM") as pspool,
        ):
            w_sb = wpool.tile([P, KS, SLOTS], mybir.dt.float16)
            nc.sync.dma_start(w_sb[:], wt.rearrange("t p r -> p t r"))

            ablate = os.environ.get("CANTOR_ABLATE", "")

            def body(_i=None):
                for si in range(NSI):
                    x_sb = xpool.tile([P, KS, NT], x_dt, tag="xld")
                    if ablate != "mm":
                        ksplit = int(os.environ.get("CANTOR_KSPLIT", "1"))
                        kh = KS // ksplit
                        for j in range(ksplit):
                            nc.sync.dma_start(
                                x_sb[:, j * kh:(j + 1) * kh],
                                xt[si, :, j * kh:(j + 1) * kh])
                    else:
                        nc.any.memset(x_sb[:], 0.0)
                    o_sb = opool.tile([P, SB, SLOTS], mybir.dt.float16,
                                      tag="o")
                    for sb in range(SB):
                        if ablate == "dma":
                            nc.any.memset(o_sb[:, sb], 0.0)
                        else:
                            # pad to 512 f32 = one full PSUM bank
                            ps = pspool.tile([P, 512], mybir.dt.float32,
                                             tag="ps")
                            for t in range(KS):
                                l, h = int(LO_T[t]), int(HI_T[t])
                                nc.tensor.matmul(
                                    ps[:, l:h],
                                    lhsT=x_sb[:, t, sb * P:(sb + 1) * P],
                                    rhs=w_sb[:, t, l:h],
                                    start=(t == 0),
                                    stop=(t == KS - 1),
                                    skip_group_check=True,
                                )
                            nc.scalar.activation(
                                o_sb[:, sb], ps[:, 0:SLOTS],
                                mybir.ActivationFunctionType.Identity)
                    nc.sync.dma_start(
                        yt[si].rearrange("sb p r -> p sb r"), o_sb[:])

            if loops == 1:
                body()
            else:
                unroll = int(os.environ.get("CANTOR_BENCH_UNROLL", "1"))
                assert loops % unroll == 0
                with tc.For_i(0, loops // unroll, 1) as i:
                    for _ in range(unroll):
                        body(i)

    nc.compile()
    return nc


def _get_nc(mm_mode, loops):
    key = (mm_mode, loops)
    if key not in _NC_CACHE:
        _NC_CACHE[key] = _build_nc(mm_mode, loops)
    return _NC_CACHE[key]


def _pack_weight(weight, mask):
    w_eff = (np.asarray(weight, np.float32)
             * np.asarray(mask, np.float32))[ROWS]     # [240, 2048]
    wt = np.zeros((KS, P, SLOTS), np.float16)
    for j, (i, s, e) in enumerate(SLOT_LIST):
        for t in range(s // P, (e - 1) // P + 1):
            a = max(s, t * P)
            b = min(e, (t + 1) * P)
            wt[t, a - t * P:b - t * P, j] = w_eff[i, a:b]
    return wt


def prep_in_maps(x, weight, bias, mask):
    x = np.asarray(x, dtype=np.float32)
    wt = _pack_weight(weight, mask)
    xf = x.reshape(S, IN_F)
    in_maps = []
    for c in range(NCORES):
        x_t = xf[c * S_SH:(c + 1) * S_SH].T.astype(ml_dtypes.float8_e3m4)
        # [IN_F, S_SH] -> [NSI, P, KS, NT]
        x_t = np.ascontiguousarray(
            x_t.reshape(KS, P, NSI, NT).transpose(2, 1, 0, 3))
        in_maps.append({XNAME: x_t, "wt": wt})
    return in_maps, ROWS


def kernel(x, weight, bias, mask):
    global LAST_RESULTS
    bias = np.asarray(bias, dtype=np.float32)
    in_maps, rows = prep_in_maps(x, weight, bias, mask)

    nc = _get_nc(MM_MODE, LOOPS)
    res = run_bass_kernel_spmd(nc, in_maps, list(range(NCORES)))
    LAST_RESULTS = res

    sec = np.flatnonzero(SECOND_SLOT >= 0)
    y = np.empty((S, OUT_F), dtype=np.float32)
    y[:] = bias
    for c in range(NCORES):
        r = res.results[c]["yt"].reshape(S_SH, SLOTS).astype(np.float32)
        acc = r[:, FIRST_SLOT]
        acc[:, sec] += r[:, SECOND_SLOT[sec]]
        y[c * S_SH:(c + 1) * S_SH, rows] = acc + bias[rows]
    return y.reshape(B, SQ, OUT_F)


# revision 10
# speedup vs baseline: 1.0068x; 1.0068x over previous
"""Masked linear (CantorLinear): y = x @ (weight*mask).T + bias.

Sparse flipped-orientation kernel. The Cantor mask keeps 326 contiguous
(row, k-run) pieces across 240 of the 2048 output rows. Each piece gets one
output "slot"; slots are sorted by run midpoint so that each 128-wide
k-subtile's user slots form a tight contiguous range (sum of ranges = 1529
vs 16*336 dense).

Per matmul: lhsT (stationary) = x k-subtile [128 k, 128 seq] in fp8-e3m4
(4-bit mantissa, abs err 1.2e-2 rel vs 2e-2 budget; halves x DMA vs fp16),
rhs (moving) = packed weight [128 k, range] in fp16, PSUM [128 seq, slots]
fp32 accumulates over the 16 k-subtiles. PE cost scales with the moving
free dim = slot range, so sparsity cuts PE ~2.2x vs the dense orientation.
PSUM has_written semantics (start=True clears the whole bank; start=False
overwrites where clear) make the per-subtile column offsets legal without
an init pass. Bias and the 2-piece row sums are applied host-side.

8 cores data-parallel over the 16384 sequence positions.
"""

import os
import numpy as np
import ml_dtypes

import concourse.bacc as bacc
import concourse.mybir as mybir
import concourse.tile as tile
import concourse.bass_utils as _bu
from concourse.bass_utils import run_bass_kernel_spmd

if os.environ.get("CANTOR_LDWOPT", "0") == "1" and not getattr(_bu, "_ldw_patched", False):
    _orig_run_command = _bu.run_command

    def _patched_run_command(argv, **kw):
        argv = ["--enable-ldw-opt=true" if a == "--enable-ldw-opt=false" else a
                for a in argv]
        return _orig_run_command(argv, **kw)

    _bu.run_command = _patched_run_command
    _bu._ldw_patched = True

B, SQ = 4, 4096
IN_F = 2048
OUT_F = 2048
S = B * SQ                 # 16384 flattened sequence positions
NCORES = 8
S_SH = S // NCORES         # 2048 per core
P = 128
KS = IN_F // P             # 16 k-subtiles
NT = int(os.environ.get("CANTOR_NT", "512"))   # seq positions per x DMA tile
NSI = S_SH // NT
SB = NT // P               # seq sub-blocks (psum groups) per x tile
SLOTS = 336                # 326 real slots padded

MM_MODE = os.environ.get("CANTOR_MM_MODE", "e3m4")
LOOPS = int(os.environ.get("CANTOR_BENCH_LOOPS", "1"))

LAST_RESULTS = None
_NC_CACHE = {}


def _cantor_mask(out_dim, in_dim, depth=8):
    idx = np.arange(out_dim * in_dim, dtype=np.float64)
    x = idx / (out_dim * in_dim + 1e-9)
    valid = np.ones(x.shape, dtype=bool)
    for _ in range(depth):
        x = x * 3.0
        digit = np.floor(x)
        x = x - digit
        valid &= (digit != 1.0)
    return valid.reshape(out_dim, in_dim)


def _build_tables():
    """Slot decomposition of the mask: returns (rows, slot list sorted by
    run midpoint, per-subtile [lo, hi) slot ranges, per-row slot indices)."""
    M = _cantor_mask(OUT_F, IN_F)
    rows = np.flatnonzero(M.any(axis=1))
    slots = []
    for i, r in enumerate(rows):
        m = M[r]
        d = np.diff(np.concatenate([[0], m.view(np.int8), [0]]))
        for s, e in zip(np.flatnonzero(d == 1), np.flatnonzero(d == -1)):
            slots.append((i, int(s), int(e)))
    slots.sort(key=lambda t: t[1] + t[2])
    assert len(slots) <= SLOTS
    lo = np.zeros(KS, np.int32)
    hi = np.zeros(KS, np.int32)
    for t in range(KS):
        a, b = t * P, (t + 1) * P
        idx = [j for j, (_, s, e) in enumerate(slots) if s < b and e > a]
        lo[t], hi[t] = idx[0], idx[-1] + 1
    first = np.full(len(rows), -1, np.int64)
    second = np.full(len(rows), -1, np.int64)
    for j, (i, s, e) in enumerate(slots):
        if first[i] < 0:
            first[i] = j
        else:
            second[i] = j
    return rows, slots, lo, hi, first, second


ROWS, SLOT_LIST, LO_T, HI_T, FIRST_SLOT, SECOND_SLOT = _build_tables()


XNAME = "xtl" if os.environ.get("CANTOR_LDWOPT", "0") == "1" else "xt"


def _build_nc(mm_mode, loops):
    x_dt = mybir.dt.float8e3
    nc = bacc.Bacc("TRN2", target_bir_lowering=False, debug=False)
    xt = nc.dram_tensor(XNAME, [NSI, P, KS, NT], x_dt, kind="ExternalInput")
    wt = nc.dram_tensor("wt", [KS, P, SLOTS], mybir.dt.float16,
                        kind="ExternalInput")
    # [si][sb][p][slot] — host reshapes to [S_SH, SLOTS] (same bytes)
    yt = nc.dram_tensor("yt", [NSI, SB, P, SLOTS], mybir.dt.float16,
                        kind="ExternalOutput")

    with tile.TileContext(nc) as tc:
        with (
            tc.tile_pool(name="wpool", bufs=1) as wpool,
            tc.tile_pool(name="xpool",
                         bufs=int(os.environ.get("CANTOR_XBUFS", "3"))) as xpool,
            tc.tile_pool(name="opool",
                         bufs=int(os.environ.get("CANTOR_OBUFS", "4"))) as opool,
            tc.tile_pool(name="pspool",
                         bufs=int(os.environ.get("CANTOR_PSBUFS", "4")),
                         space="PSUM") as pspool,
        ):
            w_sb = wpool.tile([P, KS, SLOTS], mybir.dt.float16)
            nc.sync.dma_start(w_sb[:], wt.rearrange("t p r -> p t r"))

            ablate = os.environ.get("CANTOR_ABLATE", "")

            def body(_i=None):
                for si in range(NSI):
                    x_sb = xpool.tile([P, KS, NT], x_dt, tag="xld")
                    if ablate != "mm":
                        ksplit = int(os.environ.get("CANTOR_KSPLIT", "1"))
                        kh = KS // ksplit
                        for j in range(ksplit):
                            nc.sync.dma_start(
                                x_sb[:, j * kh:(j + 1) * kh],
                                xt[si, :, j * kh:(j + 1) * kh])
                    else:
                        nc.any.memset(x_sb[:], 0.0)
                    o_sb = opool.tile([P, SB, SLOTS], mybir.dt.float16,
                                      tag="o")
                    for sb in range(SB):
                        if ablate == "dma":
                            nc.any.memset(o_sb[:, sb], 0.0)
                        else:
                            # pad to 512 f32 = one full PSUM bank
                            ps = pspool.tile([P, 512], mybir.dt.float32,
                                             tag="ps")
                            for t in range(KS):
                                l, h = int(LO_T[t]), int(HI_T[t])
                                nc.tensor.matmul(
                                    ps[:, l:h],
                                    lhsT=x_sb[:, t, sb * P:(sb + 1) * P],
                                    rhs=w_sb[:, t, l:h],
                                    start=(t == 0),
                                    stop=(t == KS - 1),
                                    skip_group_check=True,
                                )
                            nc.scalar.activation(
                                o_sb[:, sb], ps[:, 0:SLOTS],
                                mybir.ActivationFunctionType.Identity)
                    yeng = (nc.scalar
                            if os.environ.get("CANTOR_YENG", "sp") == "act"
                            else nc.sync)
                    yeng.dma_start(
                        yt[si].rearrange("sb p r -> p sb r"), o_sb[:])

            if loops == 1:
                body()
            else:
                unroll = int(os.environ.get("CANTOR_BENCH_UNROLL", "1"))
                assert loops % unroll == 0
                with tc.For_i(0, loops // unroll, 1) as i:
                    for _ in range(unroll):
                        body(i)

    nc.compile()
    return nc


def _get_nc(mm_mode, loops):
    key = (mm_mode, loops)
    if key not in _NC_CACHE:
        _NC_CACHE[key] = _build_nc(mm_mode, loops)
    return _NC_CACHE[key]


def _pack_weight(weight, mask):
    w_eff = (np.asarray(weight, np.float32)
             * np.asarray(mask, np.float32))[ROWS]     # [240, 2048]
    wt = np.zeros((KS, P, SLOTS), np.float16)
    for j, (i, s, e) in enumerate(SLOT_LIST):
        for t in range(s // P, (e - 1) // P + 1):
            a = max(s, t * P)
            b = min(e, (t + 1) * P)
            wt[t, a - t * P:b - t * P, j] = w_eff[i, a:b]
    return wt


def prep_in_maps(x, weight, bias, mask):
    x = np.asarray(x, dtype=np.float32)
    wt = _pack_weight(weight, mask)
    xf = x.reshape(S, IN_F)
    in_maps = []
    for c in range(NCORES):
        x_t = xf[c * S_SH:(c + 1) * S_SH].T.astype(ml_dtypes.float8_e3m4)
        # [IN_F, S_SH] -> [NSI, P, KS, NT]
        x_t = np.ascontiguousarray(
            x_t.reshape(KS, P, NSI, NT).transpose(2, 1, 0, 3))
        in_maps.append({XNAME: x_t, "wt": wt})
    return in_maps, ROWS


def kernel(x, weight, bias, mask):
    global LAST_RESULTS
    bias = np.asarray(bias, dtype=np.float32)
    in_maps, rows = prep_in_maps(x, weight, bias, mask)

    nc = _get_nc(MM_MODE, LOOPS)
    res = run_bass_kernel_spmd(nc, in_maps, list(range(NCORES)))
    LAST_RESULTS = res

    sec = np.flatnonzero(SECOND_SLOT >= 0)
    y = np.empty((S, OUT_F), dtype=np.float32)
    y[:] = bias
    for c in range(NCORES):
        r = res.results[c]["yt"].reshape(S_SH, SLOTS).astype(np.float32)
        acc = r[:, FIRST_SLOT]
        acc[:, sec] += r[:, SECOND_SLOT[sec]]
        y[c * S_SH:(c + 1) * S_SH, rows] = acc + bias[rows]
    return y.reshape(B, SQ, OUT_F)


# revision 11
# speedup vs baseline: 1.0797x; 1.0724x over previous
"""Masked linear (CantorLinear): y = x @ (weight*mask).T + bias.

Sparse flipped-orientation kernel. The Cantor mask keeps 326 contiguous
(row, k-run) pieces across 240 of the 2048 output rows. Each piece gets one
output "slot"; slots are sorted by run midpoint so that each 128-wide
k-subtile's user slots form a tight contiguous range (sum of ranges = 1529
vs 16*336 dense).

Per matmul: lhsT (stationary) = x k-subtile [128 k, 128 seq] in fp8-e3m4
(4-bit mantissa, abs err 1.2e-2 rel vs 2e-2 budget; halves x DMA vs fp16),
rhs (moving) = packed weight [128 k, range] in fp16, PSUM [128 seq, slots]
fp32 accumulates over the 16 k-subtiles. PE cost scales with the moving
free dim = slot range, so sparsity cuts PE ~2.2x vs the dense orientation.
PSUM has_written semantics (start=True clears the whole bank; start=False
overwrites where clear) make the per-subtile column offsets legal without
an init pass. Bias and the 2-piece row sums are applied host-side.

8 cores data-parallel over the 16384 sequence positions.
"""

import os
import numpy as np
import ml_dtypes

import concourse.bacc as bacc
import concourse.mybir as mybir
import concourse.tile as tile
import concourse.bass_utils as _bu
from concourse.bass_utils import run_bass_kernel_spmd

if os.environ.get("CANTOR_LDWOPT", "0") == "1" and not getattr(_bu, "_ldw_patched", False):
    _orig_run_command = _bu.run_command

    def _patched_run_command(argv, **kw):
        argv = ["--enable-ldw-opt=true" if a == "--enable-ldw-opt=false" else a
                for a in argv]
        return _orig_run_command(argv, **kw)

    _bu.run_command = _patched_run_command
    _bu._ldw_patched = True

B, SQ = 4, 4096
IN_F = 2048
OUT_F = 2048
S = B * SQ                 # 16384 flattened sequence positions
NCORES = 8
S_SH = S // NCORES         # 2048 per core
P = 128
KS = IN_F // P             # 16 k-subtiles
NT = int(os.environ.get("CANTOR_NT", "512"))   # seq positions per x DMA tile
NSI = S_SH // NT
SB = NT // P               # seq sub-blocks (psum groups) per x tile
SLOTS = 336                # 326 real slots padded

MM_MODE = os.environ.get("CANTOR_MM_MODE", "e3m4")
LOOPS = int(os.environ.get("CANTOR_BENCH_LOOPS", "1"))

LAST_RESULTS = None
_NC_CACHE = {}


def _cantor_mask(out_dim, in_dim, depth=8):
    idx = np.arange(out_dim * in_dim, dtype=np.float64)
    x = idx / (out_dim * in_dim + 1e-9)
    valid = np.ones(x.shape, dtype=bool)
    for _ in range(depth):
        x = x * 3.0
        digit = np.floor(x)
        x = x - digit
        valid &= (digit != 1.0)
    return valid.reshape(out_dim, in_dim)


def _build_tables():
    """Slot decomposition of the mask: returns (rows, slot list sorted by
    run midpoint, per-subtile [lo, hi) slot ranges, per-row slot indices)."""
    M = _cantor_mask(OUT_F, IN_F)
    rows = np.flatnonzero(M.any(axis=1))
    slots = []
    for i, r in enumerate(rows):
        m = M[r]
        d = np.diff(np.concatenate([[0], m.view(np.int8), [0]]))
        for s, e in zip(np.flatnonzero(d == 1), np.flatnonzero(d == -1)):
            slots.append((i, int(s), int(e)))
    slots.sort(key=lambda t: t[1] + t[2])
    assert len(slots) <= SLOTS
    lo = np.zeros(KS, np.int32)
    hi = np.zeros(KS, np.int32)
    for t in range(KS):
        a, b = t * P, (t + 1) * P
        idx = [j for j, (_, s, e) in enumerate(slots) if s < b and e > a]
        lo[t], hi[t] = idx[0], idx[-1] + 1
    first = np.full(len(rows), -1, np.int64)
    second = np.full(len(rows), -1, np.int64)
    for j, (i, s, e) in enumerate(slots):
        if first[i] < 0:
            first[i] = j
        else:
            second[i] = j
    return rows, slots, lo, hi, first, second


ROWS, SLOT_LIST, LO_T, HI_T, FIRST_SLOT, SECOND_SLOT = _build_tables()


XNAME = "xtl" if os.environ.get("CANTOR_LDWOPT", "0") == "1" else "xt"


def _build_nc(mm_mode, loops):
    x_dt = mybir.dt.float8e3
    nc = bacc.Bacc("TRN2", target_bir_lowering=False, debug=False)
    xt = nc.dram_tensor(XNAME, [NSI, P, KS, NT], x_dt, kind="ExternalInput")
    wt = nc.dram_tensor("wt", [KS, P, SLOTS], mybir.dt.float16,
                        kind="ExternalInput")
    # [si][sb][p][slot] — host reshapes to [S_SH, SLOTS] (same bytes)
    yt = nc.dram_tensor("yt", [NSI, SB, P, SLOTS], mybir.dt.float16,
                        kind="ExternalOutput")

    with tile.TileContext(nc) as tc:
        with (
            tc.tile_pool(name="wpool", bufs=1) as wpool,
            tc.tile_pool(name="xpool",
                         bufs=int(os.environ.get("CANTOR_XBUFS", "3"))) as xpool,
            tc.tile_pool(name="opool",
                         bufs=int(os.environ.get("CANTOR_OBUFS", "4"))) as opool,
            tc.tile_pool(name="pspool",
                         bufs=int(os.environ.get("CANTOR_PSBUFS", "4")),
                         space="PSUM") as pspool,
        ):
            w_sb = wpool.tile([P, KS, SLOTS], mybir.dt.float16)
            nc.sync.dma_start(w_sb[:], wt.rearrange("t p r -> p t r"))

            ablate = os.environ.get("CANTOR_ABLATE", "")

            def body(_i=None):
                for si in range(NSI):
                    x_sb = xpool.tile([P, KS, NT], x_dt, tag="xld")
                    if ablate != "mm":
                        ksplit = int(os.environ.get("CANTOR_KSPLIT", "1"))
                        kh = KS // ksplit
                        for j in range(ksplit):
                            nc.sync.dma_start(
                                x_sb[:, j * kh:(j + 1) * kh],
                                xt[si, :, j * kh:(j + 1) * kh])
                    else:
                        nc.any.memset(x_sb[:], 0.0)
                    if ablate == "xdma":
                        continue
                    o_sb = opool.tile([P, SB, SLOTS], mybir.dt.float16,
                                      tag="o")
                    for sb in range(SB):
                        if ablate == "dma":
                            nc.any.memset(o_sb[:, sb], 0.0)
                        else:
                            # pad to 512 f32 = one full PSUM bank
                            ps = pspool.tile([P, 512], mybir.dt.float32,
                                             tag="ps")
                            for t in range(KS):
                                l, h = int(LO_T[t]), int(HI_T[t])
                                nc.tensor.matmul(
                                    ps[:, l:h],
                                    lhsT=x_sb[:, t, sb * P:(sb + 1) * P],
                                    rhs=w_sb[:, t, l:h],
                                    start=(t == 0),
                                    stop=(t == KS - 1),
                                    skip_group_check=True,
                                )
                            nc.scalar.activation(
                                o_sb[:, sb], ps[:, 0:SLOTS],
                                mybir.ActivationFunctionType.Identity)
                    yeng = (nc.scalar
                            if os.environ.get("CANTOR_YENG", "sp") == "act"
                            else nc.sync)
                    yeng.dma_start(
                        yt[si].rearrange("sb p r -> p sb r"), o_sb[:])

            if loops == 1:
                body()
            else:
                unroll = int(os.environ.get("CANTOR_BENCH_UNROLL", "1"))
                assert loops % unroll == 0
                with tc.For_i(0, loops // unroll, 1) as i:
                    for _ in range(unroll):
                        body(i)

    nc.compile()
    return nc


def _get_nc(mm_mode, loops):
    key = (mm_mode, loops)
    if key not in _NC_CACHE:
        _NC_CACHE[key] = _build_nc(mm_mode, loops)
    return _NC_CACHE[key]


def _pack_weight(weight, mask):
    w_eff = (np.asarray(weight, np.float32)
             * np.asarray(mask, np.float32))[ROWS]     # [240, 2048]
    wt = np.zeros((KS, P, SLOTS), np.float16)
    for j, (i, s, e) in enumerate(SLOT_LIST):
        for t in range(s // P, (e - 1) // P + 1):
            a = max(s, t * P)
            b = min(e, (t + 1) * P)
            wt[t, a - t * P:b - t * P, j] = w_eff[i, a:b]
    return wt


def prep_in_maps(x, weight, bias, mask):
    x = np.asarray(x, dtype=np.float32)
    wt = _pack_weight(weight, mask)
    xf = x.reshape(S, IN_F)
    in_maps = []
    for c in range(NCORES):
        x_t = xf[c * S_SH:(c + 1) * S_SH].T.astype(ml_dtypes.float8_e3m4)
        # [IN_F, S_SH] -> [NSI, P, KS, NT]
        x_t = np.ascontiguousarray(
            x_t.reshape(KS, P, NSI, NT).transpose(2, 1, 0, 3))
        in_maps.append({XNAME: x_t, "wt": wt})
    return in_maps, ROWS


def kernel(x, weight, bias, mask):
    global LAST_RESULTS
    bias = np.asarray(bias, dtype=np.float32)
    in_maps, rows = prep_in_maps(x, weight, bias, mask)

    nc = _get_nc(MM_MODE, LOOPS)
    res = run_bass_kernel_spmd(nc, in_maps, list(range(NCORES)))
    LAST_RESULTS = res

    sec = np.flatnonzero(SECOND_SLOT >= 0)
    y = np.empty((S, OUT_F), dtype=np.float32)
    y[:] = bias
    for c in range(NCORES):
        r = res.results[c]["yt"].reshape(S_SH, SLOTS).astype(np.float32)
        acc = r[:, FIRST_SLOT]
        acc[:, sec] += r[:, SECOND_SLOT[sec]]
        y[c * S_SH:(c + 1) * S_SH, rows] = acc + bias[rows]
    return y.reshape(B, SQ, OUT_F)


# revision 12
# speedup vs baseline: 1.1167x; 1.0343x over previous
"""Masked linear (CantorLinear): y = x @ (weight*mask).T + bias.

Sparse flipped-orientation kernel. The Cantor mask keeps 326 contiguous
(row, k-run) pieces across 240 of the 2048 output rows. Each piece gets one
output "slot"; slots are sorted by run midpoint so that each 128-wide
k-subtile's user slots form a tight contiguous range (sum of ranges = 1529
vs 16*336 dense).

Per matmul: lhsT (stationary) = x k-subtile [128 k, 128 seq] in fp8-e3m4
(4-bit mantissa, abs err 1.2e-2 rel vs 2e-2 budget; halves x DMA vs fp16),
rhs (moving) = packed weight [128 k, range] in fp16, PSUM [128 seq, slots]
fp32 accumulates over the 16 k-subtiles. PE cost scales with the moving
free dim = slot range, so sparsity cuts PE ~2.2x vs the dense orientation.
PSUM has_written semantics (start=True clears the whole bank; start=False
overwrites where clear) make the per-subtile column offsets legal without
an init pass. Bias and the 2-piece row sums are applied host-side.

8 cores data-parallel over the 16384 sequence positions.
"""

import os
import numpy as np
import ml_dtypes

import concourse.bacc as bacc
import concourse.mybir as mybir
import concourse.tile as tile
import concourse.bass_utils as _bu
from concourse.bass_utils import run_bass_kernel_spmd

if os.environ.get("CANTOR_LDWOPT", "0") == "1" and not getattr(_bu, "_ldw_patched", False):
    _orig_run_command = _bu.run_command

    def _patched_run_command(argv, **kw):
        argv = ["--enable-ldw-opt=true" if a == "--enable-ldw-opt=false" else a
                for a in argv]
        return _orig_run_command(argv, **kw)

    _bu.run_command = _patched_run_command
    _bu._ldw_patched = True

B, SQ = 4, 4096
IN_F = 2048
OUT_F = 2048
S = B * SQ                 # 16384 flattened sequence positions
NCORES = 8
S_SH = S // NCORES         # 2048 per core
P = 128
KS = IN_F // P             # 16 k-subtiles
NT = int(os.environ.get("CANTOR_NT", "2048"))   # seq positions per x DMA tile
NSI = S_SH // NT
SB = NT // P               # seq sub-blocks (psum groups) per x tile
SLOTS = 336                # 326 real slots padded

MM_MODE = os.environ.get("CANTOR_MM_MODE", "e3m4")
LOOPS = int(os.environ.get("CANTOR_BENCH_LOOPS", "1"))

LAST_RESULTS = None
_NC_CACHE = {}


def _cantor_mask(out_dim, in_dim, depth=8):
    idx = np.arange(out_dim * in_dim, dtype=np.float64)
    x = idx / (out_dim * in_dim + 1e-9)
    valid = np.ones(x.shape, dtype=bool)
    for _ in range(depth):
        x = x * 3.0
        digit = np.floor(x)
        x = x - digit
        valid &= (digit != 1.0)
    return valid.reshape(out_dim, in_dim)


def _build_tables():
    """Slot decomposition of the mask: returns (rows, slot list sorted by
    run midpoint, per-subtile [lo, hi) slot ranges, per-row slot indices)."""
    M = _cantor_mask(OUT_F, IN_F)
    rows = np.flatnonzero(M.any(axis=1))
    slots = []
    for i, r in enumerate(rows):
        m = M[r]
        d = np.diff(np.concatenate([[0], m.view(np.int8), [0]]))
        for s, e in zip(np.flatnonzero(d == 1), np.flatnonzero(d == -1)):
            slots.append((i, int(s), int(e)))
    slots.sort(key=lambda t: t[1] + t[2])
    assert len(slots) <= SLOTS
    lo = np.zeros(KS, np.int32)
    hi = np.zeros(KS, np.int32)
    for t in range(KS):
        a, b = t * P, (t + 1) * P
        idx = [j for j, (_, s, e) in enumerate(slots) if s < b and e > a]
        lo[t], hi[t] = idx[0], idx[-1] + 1
    first = np.full(len(rows), -1, np.int64)
    second = np.full(len(rows), -1, np.int64)
    for j, (i, s, e) in enumerate(slots):
        if first[i] < 0:
            first[i] = j
        else:
            second[i] = j
    return rows, slots, lo, hi, first, second


ROWS, SLOT_LIST, LO_T, HI_T, FIRST_SLOT, SECOND_SLOT = _build_tables()


XNAME = "xtl" if os.environ.get("CANTOR_LDWOPT", "0") == "1" else "xt"


def _build_nc(mm_mode, loops):
    x_dt = mybir.dt.float8e3
    nc = bacc.Bacc("TRN2", target_bir_lowering=False, debug=False)
    xt = nc.dram_tensor(XNAME, [NSI, P, KS, NT], x_dt, kind="ExternalInput")
    wt = nc.dram_tensor("wt", [KS, P, SLOTS], mybir.dt.float16,
                        kind="ExternalInput")
    # [si][sb][p][slot] — host reshapes to [S_SH, SLOTS] (same bytes)
    yt = nc.dram_tensor("yt", [NSI, SB, P, SLOTS], mybir.dt.float16,
                        kind="ExternalOutput")

    with tile.TileContext(nc) as tc:
        with (
            tc.tile_pool(name="wpool", bufs=1) as wpool,
            tc.tile_pool(name="xpool",
                         bufs=int(os.environ.get("CANTOR_XBUFS", "3"))) as xpool,
            tc.tile_pool(name="opool",
                         bufs=int(os.environ.get("CANTOR_OBUFS", "4"))) as opool,
            tc.tile_pool(name="pspool",
                         bufs=int(os.environ.get("CANTOR_PSBUFS", "6")),
                         space="PSUM") as pspool,
        ):
            w_sb = wpool.tile([P, KS, SLOTS], mybir.dt.float16)
            nc.sync.dma_start(w_sb[:], wt.rearrange("t p r -> p t r"))

            ablate = os.environ.get("CANTOR_ABLATE", "")

            def body(_i=None):
                for si in range(NSI):
                    x_sb = xpool.tile([P, KS, NT], x_dt, tag="xld")
                    if ablate != "mm":
                        ksplit = int(os.environ.get("CANTOR_KSPLIT", "2"))
                        kh = KS // ksplit
                        for j in range(ksplit):
                            nc.sync.dma_start(
                                x_sb[:, j * kh:(j + 1) * kh],
                                xt[si, :, j * kh:(j + 1) * kh])
                    else:
                        nc.any.memset(x_sb[:], 0.0)
                    if ablate == "xdma":
                        continue
                    o_sb = opool.tile([P, SB, SLOTS], mybir.dt.float16,
                                      tag="o")
                    for sb in range(SB):
                        if ablate == "dma":
                            nc.any.memset(o_sb[:, sb], 0.0)
                        else:
                            # pad to 512 f32 = one full PSUM bank
                            ps = pspool.tile([P, 512], mybir.dt.float32,
                                             tag="ps")
                            for t in range(KS):
                                l, h = int(LO_T[t]), int(HI_T[t])
                                nc.tensor.matmul(
                                    ps[:, l:h],
                                    lhsT=x_sb[:, t, sb * P:(sb + 1) * P],
                                    rhs=w_sb[:, t, l:h],
                                    start=(t == 0),
                                    stop=(t == KS - 1),
                                    skip_group_check=True,
                                )
                            nc.scalar.activation(
                                o_sb[:, sb], ps[:, 0:SLOTS],
                                mybir.ActivationFunctionType.Identity)
                    yeng = (nc.scalar
                            if os.environ.get("CANTOR_YENG", "sp") == "act"
                            else nc.sync)
                    yeng.dma_start(
                        yt[si].rearrange("sb p r -> p sb r"), o_sb[:])

            if loops == 1:
                body()
            else:
                unroll = int(os.environ.get("CANTOR_BENCH_UNROLL", "1"))
                assert loops % unroll == 0
                with tc.For_i(0, loops // unroll, 1) as i:
                    for _ in range(unroll):
                        body(i)

    nc.compile()
    return nc


def _get_nc(mm_mode, loops):
    key = (mm_mode, loops)
    if key not in _NC_CACHE:
        _NC_CACHE[key] = _build_nc(mm_mode, loops)
    return _NC_CACHE[key]


def _pack_weight(weight, mask):
    w_eff = (np.asarray(weight, np.float32)
             * np.asarray(mask, np.float32))[ROWS]     # [240, 2048]
    wt = np.zeros((KS, P, SLOTS), np.float16)
    for j, (i, s, e) in enumerate(SLOT_LIST):
        for t in range(s // P, (e - 1) // P + 1):
            a = max(s, t * P)
            b = min(e, (t + 1) * P)
            wt[t, a - t * P:b - t * P, j] = w_eff[i, a:b]
    return wt


def prep_in_maps(x, weight, bias, mask):
    x = np.asarray(x, dtype=np.float32)
    wt = _pack_weight(weight, mask)
    xf = x.reshape(S, IN_F)
    in_maps = []
    for c in range(NCORES):
        x_t = xf[c * S_SH:(c + 1) * S_SH].T.astype(ml_dtypes.float8_e3m4)
        # [IN_F, S_SH] -> [NSI, P, KS, NT]
        x_t = np.ascontiguousarray(
            x_t.reshape(KS, P, NSI, NT).transpose(2, 1, 0, 3))
        in_maps.append({XNAME: x_t, "wt": wt})
    return in_maps, ROWS


def kernel(x, weight, bias, mask):
    global LAST_RESULTS
    bias = np.asarray(bias, dtype=np.float32)
    in_maps, rows = prep_in_maps(x, weight, bias, mask)

    nc = _get_nc(MM_MODE, LOOPS)
    res = run_bass_kernel_spmd(nc, in_maps, list(range(NCORES)))
    LAST_RESULTS = res

    sec = np.flatnonzero(SECOND_SLOT >= 0)
    y = np.empty((S, OUT_F), dtype=np.float32)
    y[:] = bias
    for c in range(NCORES):
        r = res.results[c]["yt"].reshape(S_SH, SLOTS).astype(np.float32)
        acc = r[:, FIRST_SLOT]
        acc[:, sec] += r[:, SECOND_SLOT[sec]]
        y[c * S_SH:(c + 1) * S_SH, rows] = acc + bias[rows]
    return y.reshape(B, SQ, OUT_F)
